# revision 1
# baseline (speedup 1.0000x reference)
"""Conv2dfft kernel for Trainium2 (8 NeuronCores, SPMD data-parallel over N).

The reference computes an FFT-based 2D cross-correlation that is exactly a
3x3 same-padding conv2d: out[n,f,h,w] = sum_{c,ky,kx} x[n,c,h+ky-1,w+kx-1]
* weight[f,c,ky,kx] + bias[f]  (zero-padded at the borders).

We implement it directly as 9 shifted 128x128 matmuls accumulated in PSUM:
the contraction dim C=128 fills the PE partition dim, F=128 fills the output
partition dim. Data-parallel: 32 images / 8 cores = 4 images per core.
The input is zero-padded to 34x34 on the host so every tap is a full, even,
aligned 512-element FP32r matmul (ISA fp32r restrictions) and the x DMA is
fully contiguous. Each padded image is loaded as two overlapping 18-row
chunks so compute on the first half starts as soon as it lands, and warmup
matmuls on the weight tile bridge the input-DMA latency while warming the
PE HAM clock gate.
"""

import numpy as np

import concourse.bass as bass
import concourse.tile as tile
from concourse import bacc, mybir
from concourse.bass_utils import run_bass_kernel_spmd

N, C, F, H, W = 32, 128, 128, 32, 32
N_CORES = 8
N_LOC = N // N_CORES  # images per core
HP, WP = H + 2, W + 2  # host-padded image
HB = 16      # rows per PSUM block (16*32 = 512 = one PSUM bank)
HC = HB + 2  # rows per x chunk (chunk hb covers padded rows 16*hb .. +18)
N_WARM = 3   # warmup matmuls on the weight tile

F32 = mybir.dt.float32
F32R = mybir.dt.float32r


def _light_drain_and_barrier(self, tick_clock, wait_clock):
    """Tile epilogue without the trailing all-engine barrier.

    Nothing executes after the semaphore clears inside this kernel, and the
    runtime won't re-dispatch the NEFF until every engine queue has drained,
    so the final barrier only adds tail latency.
    """
    from concourse.vector_clock import ScopedClock

    drain_inst = self.nc.sync.drain()
    wait_clock.add_sem_waits(
        drain_inst.ins, ScopedClock({None: tick_clock.global_clock})
    )
    self.nc.all_engine_barrier()
    popped = self.nc._tile_sem_poison_stack.pop()
    assert popped is self._sem_poison
    self.nc.clear_and_free_semaphores(list(self.sems.allocated().values()))


def _build_module():
    nc = bacc.Bacc(None, dynamic_dma_scratch_size=256)

    # x is stored as per-block 18-row chunks (rows 16*hb .. 16*hb+18 of the
    # padded image), so each (image, hb) PSUM block reads exactly one chunk.
    x_d = nc.dram_tensor(
        "x", [N_LOC, H // HB, C, HC, WP], F32R, kind="ExternalInput"
    )
    w_d = nc.dram_tensor("w", [C, 9 * F], F32R, kind="ExternalInput")
    b_d = nc.dram_tensor("b", [F, 1], F32, kind="ExternalInput")
    o_d = nc.dram_tensor("out", [N_LOC, F, H, W], F32, kind="ExternalOutput")

    n_blocks = N_LOC * (H // HB)

    tile.TileContext._drain_and_barrier = _light_drain_and_barrier
    with tile.TileContext(nc) as tc:
        with (
            tc.tile_pool(name="const", bufs=1) as cpool,
            tc.tile_pool(name="x", bufs=n_blocks) as xpool,
            tc.tile_pool(name="o", bufs=n_blocks) as opool,
            tc.tile_pool(name="ps", bufs=8, space=bass.MemorySpace.PSUM) as ppool,
        ):
            w_sb = cpool.tile([C, 9 * F], F32R)
            nc.sync.dma_start(w_sb[:], w_d[:])
            b_sb = cpool.tile([F, 1], F32)
            nc.sync.dma_start(b_sb[:], b_d[:])

            # Warmup matmuls on the weight tile: bridge the x-DMA latency,
            # warm the PE HAM clock gate, and make the PE observe the
            # weight-DMA semaphore before the first real matmul. Emitted
            # BEFORE the x DMA issues so their semaphore wait covers only
            # the weight DMA.
            ps_warm = ppool.tile([F, HB, W], F32, tag="ps")
            prev_mm = None
            for i in range(N_WARM):
                prev_mm = nc.tensor.matmul(
                    ps_warm[:],
                    w_sb[:, 0:F],
                    w_sb[:, (i % 2) * HB * W // 2 : (i % 2) * HB * W // 2 + HB * W],
                    start=True,
                    stop=True,
                )

            x_sbs = []
            for n in range(N_LOC):
                for hb in range(H // HB):
                    x_sb = xpool.tile([C, HC, WP], F32R, tag="x")
                    nc.sync.dma_start(x_sb[:], x_d[n, hb])
                    x_sbs.append(x_sb)

            for n in range(N_LOC):
                for hb in range(H // HB):
                    x_sb = x_sbs[n * (H // HB) + hb]
                    ps = ppool.tile([F, HB, W], F32, tag="ps")
                    for i, (ky, kx) in enumerate(
                        [(ky, kx) for ky in range(3) for kx in range(3)]
                    ):
                        rhs = x_sb[:, ky : ky + HB, kx : kx + W]
                        lhsT = w_sb[:, (ky * 3 + kx) * F : (ky * 3 + kx + 1) * F]
                        mm = nc.tensor.matmul(
                            ps[:],
                            lhsT,
                            rhs,
                            start=(i == 0),
                            stop=(i == 8),
                        )
                        if prev_mm is not None:
                            # keep PE issue order = program order
                            tile.add_dep_helper(
                                mm.ins, prev_mm.ins, sync=False,
                                reason="PE program order",
                            )
                        prev_mm = mm
                    # bias add PSUM -> SBUF, then store this block
                    o_sb = opool.tile([F, HB, W], F32, tag="o")
                    nc.vector.tensor_scalar_add(o_sb[:], ps[:], b_sb[:, 0:1])
                    nc.sync.dma_start(o_d[n][:, hb * HB : hb * HB + HB, :], o_sb[:])
    nc.compile()
    return nc


_NC_CACHE = None


def _tf32_round(a: np.ndarray) -> np.ndarray:
    """Round fp32 -> tf32 (10-bit mantissa), round-to-nearest-even.

    FP32r matmul inputs must arrive pre-rounded (the BIR verifier enforces
    that producers of FP32r matmul operands emit float32r); rounding on the
    host keeps the device data path a pure copy.
    """
    u = np.ascontiguousarray(a, dtype=np.float32).view(np.uint32)
    r = (u + 0x00000FFF + ((u >> 13) & 1)) & np.uint32(0xFFFFE000)
    return r.view(np.float32)


def _run(x, weight, bias, **kwargs):
    global _NC_CACHE
    if _NC_CACHE is None:
        _NC_CACHE = _build_module()
    nc = _NC_CACHE

    xp = np.zeros((N, C, HP, WP), dtype=np.float32)
    xp[:, :, 1 : 1 + H, 1 : 1 + W] = _tf32_round(np.asarray(x, dtype=np.float32))
    # per-block 18-row chunks: chunk hb = padded rows 16*hb .. 16*hb+18
    xc = np.stack([xp[:, :, 0:HC, :], xp[:, :, HB : HB + HC, :]], axis=1)
    # lhsT layout: w_pack[c, (ky*3+kx)*F + f] = weight[f, c, ky, kx]
    w_pack = _tf32_round(
        np.ascontiguousarray(
            np.asarray(weight, dtype=np.float32).transpose(1, 2, 3, 0).reshape(C, 9 * F)
        )
    )
    b2 = np.ascontiguousarray(np.asarray(bias, dtype=np.float32).reshape(F, 1))

    shards = xc.reshape(N_CORES, N_LOC, H // HB, C, HC, WP)
    in_maps = [{"x": shards[i], "w": w_pack, "b": b2} for i in range(N_CORES)]
    return run_bass_kernel_spmd(nc, in_maps, core_ids=list(range(N_CORES)), **kwargs)


def kernel(x: np.ndarray, weight: np.ndarray, bias: np.ndarray, **_) -> np.ndarray:
    res = _run(x, weight, bias)
    return np.concatenate([res.results[i]["out"] for i in range(N_CORES)], axis=0)



# revision 3
# speedup vs baseline: 1.0362x; 1.0362x over previous
"""Conv2dfft kernel for Trainium2 (8 NeuronCores, SPMD data-parallel over N).

The reference computes an FFT-based 2D cross-correlation that is exactly a
3x3 same-padding conv2d: out[n,f,h,w] = sum_{c,ky,kx} x[n,c,h+ky-1,w+kx-1]
* weight[f,c,ky,kx] + bias[f]  (zero-padded at the borders).

We implement it directly as 9 shifted 128x128 matmuls accumulated in PSUM:
the contraction dim C=128 fills the PE partition dim, F=128 fills the output
partition dim. Data-parallel: 32 images / 8 cores = 4 images per core.

v2 notes (from NTFF trace analysis of the fp32r baseline):
- Inputs are cast to bf16 on the host: the PE runs bf16 at the same
  1 column/cycle as fp32r, but the input DMA bytes halve, so the first
  real matmul can start ~1.5us earlier.  Accumulation stays fp32 in PSUM;
  measured rel err ~1e-3 vs the 2e-2 gate.
- Warmup matmuls read an *uninitialized* SBUF tile, so they have no
  semaphore dependencies at all: the PE starts ~2.5us before the weight
  DMA lands, which both hides the input DMA latency and ramps the PE
  clock (1.2GHz -> 2.4GHz after ~3-4us of sustained work) before the
  real matmuls issue.
- DMA issue order w -> x0 -> b -> x1.. so the first block's operands are
  first in the (serial) SP trigger queue; the bias only has to beat the
  first bias-add, not the first matmul.
- The module declares only the SP HWDGE dynamic-DMA queue group (8
  queues) instead of bass's default 3 groups x 16: NRT's per-execution
  queue setup/semaphore-reset work scales with what the NEFF declares.
"""

import numpy as np
import ml_dtypes

import concourse.bass as bass
import concourse.tile as tile
from concourse import bacc, mybir
from concourse.bass_utils import run_bass_kernel_spmd

N, C, F, H, W = 32, 128, 128, 32, 32
N_CORES = 8
N_LOC = N // N_CORES  # images per core
HP, WP = H + 2, W + 2  # host-padded image
HB = 16      # rows per PSUM block (16*32 = 512 = one PSUM bank)
HC = HB + 2  # rows per x chunk (chunk hb covers padded rows 16*hb .. +18)
N_WARM = 5   # dependency-free warmup matmuls (PE clock ramp + DMA bridge)

F32 = mybir.dt.float32
BF16 = mybir.dt.bfloat16


def _light_drain_and_barrier(self, tick_clock, wait_clock):
    """Tile epilogue without the trailing all-engine barrier.

    Nothing executes after the semaphore clears inside this kernel, and the
    runtime won't re-dispatch the NEFF until every engine queue has drained,
    so the final barrier only adds tail latency.
    """
    from concourse.vector_clock import ScopedClock

    drain_inst = self.nc.sync.drain()
    wait_clock.add_sem_waits(
        drain_inst.ins, ScopedClock({None: tick_clock.global_clock})
    )
    self.nc.all_engine_barrier()
    popped = self.nc._tile_sem_poison_stack.pop()
    assert popped is self._sem_poison
    self.nc.clear_and_free_semaphores(list(self.sems.allocated().values()))


def _build_module():
    nc = bacc.Bacc(None, dynamic_dma_scratch_size=256)

    # x is stored as per-block 18-row chunks (rows 16*hb .. 16*hb+18 of the
    # padded image), so each (image, hb) PSUM block reads exactly one chunk.
    x_d = nc.dram_tensor(
        "x", [N_LOC, H // HB, C, HC, WP], BF16, kind="ExternalInput"
    )
    w_d = nc.dram_tensor("w", [C, 9 * F], BF16, kind="ExternalInput")
    b_d = nc.dram_tensor("b", [F, 1], F32, kind="ExternalInput")
    o_d = nc.dram_tensor("out", [N_LOC, F, H, W], F32, kind="ExternalOutput")

    n_blocks = N_LOC * (H // HB)

    tile.TileContext._drain_and_barrier = _light_drain_and_barrier
    with tile.TileContext(nc) as tc:
        with (
            tc.tile_pool(name="const", bufs=1) as cpool,
            tc.tile_pool(name="x", bufs=n_blocks) as xpool,
            tc.tile_pool(name="o", bufs=n_blocks) as opool,
            tc.tile_pool(name="ps", bufs=8, space=bass.MemorySpace.PSUM) as ppool,
        ):
            # DMA-free warmups: matmul on a gpsimd-memset SBUF tile. No DMA
            # producer -> the PE starts as soon as gpsimd's cheap memset
            # lands, ramping the clock while the input DMAs fly.
            wu = cpool.tile([C, 512], BF16)
            nc.gpsimd.memset(wu[:], 0)
            ps_warm = ppool.tile([F, HB, W], F32, tag="ps")
            prev_mm = None
            for i in range(N_WARM):
                prev_mm = nc.tensor.matmul(
                    ps_warm[:],
                    wu[:, 0:F],
                    wu[:],
                    start=True,
                    stop=True,
                )

            w_sb = cpool.tile([C, 9 * F], BF16)
            nc.sync.dma_start(w_sb[:], w_d[:])

            # x chunk 0 right behind w; bias after x0 (it only has to beat
            # the first bias-add, several us after the first matmul).
            x_sbs = []
            x_sb = xpool.tile([C, HC, WP], BF16, tag="x")
            nc.sync.dma_start(x_sb[:], x_d[0, 0])
            x_sbs.append(x_sb)

            b_sb = cpool.tile([F, 1], F32)
            nc.sync.dma_start(b_sb[:], b_d[:])

            for blk in range(1, n_blocks):
                n, hb = divmod(blk, H // HB)
                x_sb = xpool.tile([C, HC, WP], BF16, tag="x")
                nc.sync.dma_start(x_sb[:], x_d[n, hb])
                x_sbs.append(x_sb)

            for n in range(N_LOC):
                for hb in range(H // HB):
                    x_sb = x_sbs[n * (H // HB) + hb]
                    ps = ppool.tile([F, HB, W], F32, tag="ps")
                    for i, (ky, kx) in enumerate(
                        [(ky, kx) for ky in range(3) for kx in range(3)]
                    ):
                        rhs = x_sb[:, ky : ky + HB, kx : kx + W]
                        lhsT = w_sb[:, (ky * 3 + kx) * F : (ky * 3 + kx + 1) * F]
                        mm = nc.tensor.matmul(
                            ps[:],
                            lhsT,
                            rhs,
                            start=(i == 0),
                            stop=(i == 8),
                        )
                        if prev_mm is not None:
                            # keep PE issue order = program order
                            tile.add_dep_helper(
                                mm.ins, prev_mm.ins, sync=False,
                                reason="PE program order",
                            )
                        prev_mm = mm
                    # bias add PSUM -> SBUF, then store this block
                    o_sb = opool.tile([F, HB, W], F32, tag="o")
                    nc.vector.tensor_scalar_add(o_sb[:], ps[:], b_sb[:, 0:1])
                    nc.sync.dma_start(o_d[n][:, hb * HB : hb * HB + HB, :], o_sb[:])
    nc.compile()

    # Declare only the DMA queue group we use (SP HWDGE), and fewer rings:
    # NRT's per-execution queue init/reset work scales with declarations.
    nc.m.queues = [q for q in nc.m.queues if q.name == "qSPDynamicHW"]
    for q in nc.m.queues:
        q.num_queues = 8
    return nc


_NC_CACHE = None


def _run(x, weight, bias, **kwargs):
    global _NC_CACHE
    if _NC_CACHE is None:
        _NC_CACHE = _build_module()
    nc = _NC_CACHE

    xp = np.zeros((N, C, HP, WP), dtype=ml_dtypes.bfloat16)
    xp[:, :, 1 : 1 + H, 1 : 1 + W] = np.asarray(x, dtype=np.float32).astype(
        ml_dtypes.bfloat16
    )
    # per-block 18-row chunks: chunk hb = padded rows 16*hb .. 16*hb+18
    xc = np.stack([xp[:, :, 0:HC, :], xp[:, :, HB : HB + HC, :]], axis=1)
    # lhsT layout: w_pack[c, (ky*3+kx)*F + f] = weight[f, c, ky, kx]
    w_pack = np.ascontiguousarray(
        np.asarray(weight, dtype=np.float32).transpose(1, 2, 3, 0).reshape(C, 9 * F)
    ).astype(ml_dtypes.bfloat16)
    b2 = np.ascontiguousarray(np.asarray(bias, dtype=np.float32).reshape(F, 1))

    shards = xc.reshape(N_CORES, N_LOC, H // HB, C, HC, WP)
    in_maps = [{"x": shards[i], "w": w_pack, "b": b2} for i in range(N_CORES)]
    return run_bass_kernel_spmd(nc, in_maps, core_ids=list(range(N_CORES)), **kwargs)


def kernel(x: np.ndarray, weight: np.ndarray, bias: np.ndarray, **_) -> np.ndarray:
    res = _run(x, weight, bias)
    return np.concatenate([res.results[i]["out"] for i in range(N_CORES)], axis=0)
